# revision 1
# baseline (speedup 1.0000x reference)
"""Additive (Bahdanau) attention on 8 Trainium2 cores.

Math: scores[b,q,k] = sum_e vT[e] * tanh(qp[b,q,e] + kp[b,k,e]);
out = softmax_k(scores) @ value.  qp = query @ Wq^T, kp = key @ Wk^T.

tanh(z) ~ c*z + sum_{m=1..4} b_m sin(m*pi*z/L) on the data range (|z|<7.5,
std ~1.17).  The angle-addition identity factorizes each harmonic into
separable q/k products -> TensorE matmuls contracting over E.  All four
harmonics derive from a SINGLE sin/cos pair per side (2 ScalarE Sins per
side total) via double/triple-angle product identities evaluated on
DVE/Pool:
    sin2 = 2 s c, cos2 = 1-2s^2, sin3 = s(3-4s^2), cos3 = c(1-4s^2),
    sin4 = 4(sc)(1-2s^2), cos4 = 1-8(sc)^2.
The linear term's q-part is softmax-invariant (dropped); its k-part is
exact via exp(bias_k)-scaled value rows: softmax(s+bias) = p*e^bias
renormalized, with the denominator obtained from a 129th all-w column in
the AV matmul.  No max-subtraction pass (scores are bounded ~|2.3|; exp
input shifted by -6 for spline-range safety).

Scores are built TRANSPOSED ([k-tile, q]) so softmax probabilities come
out of the Exp activation already in the layout the AV matmul needs --
no per-tile probability transposes or PSUM->SBUF copies.

Measured output relative error vs the fp32 reference: ~3.6e-3.

Sharding: core = (batch, q-block): 2 batches x 4 q-blocks of 256 rows.
Each core computes its full attention independently; no collectives.
"""

import numpy as np

import concourse.bass as bass
import concourse.tile as tile
from concourse import mybir
from concourse.bass_utils import run_bass_kernel_spmd
from concourse.masks import make_identity

F32 = mybir.dt.float32
F32R = mybir.dt.float32r
F16 = mybir.dt.float16
AF = mybir.ActivationFunctionType
ALU = mybir.AluOpType

# ---- problem shapes (hardcoded per contract) ----
B, LQ, LK, D, E, VD = 2, 1024, 1024, 128, 128, 128
N_CORES = 8
QSH = (B * LQ) // N_CORES          # 256 q rows per core
NKT = LK // 128                    # 8 k-tiles of 128
NBLK = LK // 512                   # 2 k feature blocks of 512

# ---- tanh approximation: c*z + sum b_m sin(m*pi*z/L), fit on the data
# distribution (Gaussian sigma~1.17 bulk + uniform floor to 7.65) ----
L_PER = 4.7
U_SCALE = float(1.0 / (2.0 * L_PER))   # angle in period units: u = z/(2L)
C_LIN = 0.21141849494658685
B1 = 0.5361037181838887
B2 = 0.1670207745739602
B3 = 0.05380947955713597
B4 = 0.02681091036953445
TWO_PI = float(np.float32(2 * np.pi))
ANG = float(np.float32(2 * np.pi / (2.0 * L_PER)))   # z -> sin angle, U folded
HALF_PI = float(np.float32(np.pi / 2))
EXP_SHIFT = -6.0


def build_nc():
    nc = bass.Bass("TRN2", target_bir_lowering=False, debug=False)

    q_d = nc.dram_tensor("q", [QSH, D], F32, kind="ExternalInput").ap()
    k_d = nc.dram_tensor("k", [LK, D], F32, kind="ExternalInput").ap()
    v_d = nc.dram_tensor("v", [LK, VD], F32, kind="ExternalInput").ap()
    w_d = nc.dram_tensor("w", [E, 2 * D], F32, kind="ExternalInput").ap()
    vt_d = nc.dram_tensor("vt", [E, 1], F32, kind="ExternalInput").ap()
    out_d = nc.dram_tensor("out", [QSH, VD], F32, kind="ExternalOutput").ap()

    with tile.TileContext(nc) as tc:
        _body(tc, q_d, k_d, v_d, w_d, vt_d, out_d)
    return nc


def _body(tc, q_d, k_d, v_d, w_d, vt_d, out_d):
    nc = tc.nc
    from contextlib import ExitStack
    ctx = ExitStack()
    with ctx:
        const = ctx.enter_context(tc.tile_pool(name="const", bufs=1))
        kG = ctx.enter_context(tc.tile_pool(name="kG", bufs=1))
        kraw = ctx.enter_context(tc.tile_pool(name="kraw", bufs=2))
        qprod = ctx.enter_context(tc.tile_pool(name="qprod", bufs=1))
        probs_p = ctx.enter_context(tc.tile_pool(name="probs", bufs=4))
        outp = ctx.enter_context(tc.tile_pool(name="outp", bufs=1))
        stat = ctx.enter_context(tc.tile_pool(name="stat", bufs=2))
        ps_sc = ctx.enter_context(tc.tile_pool(name="ps_sc", bufs=4, space="PSUM"))
        ps_set = ctx.enter_context(tc.tile_pool(name="ps_set", bufs=2, space="PSUM"))
        ps_sm = ctx.enter_context(tc.tile_pool(name="ps_sm", bufs=1, space="PSUM"))

        # Sin's float bias must be a pre-registered const AP (sundagen only
        # accepts immediate bias for Copy/Reciprocal); tile-tracked so the
        # memset->read dependency is synced without an all-engine barrier.
        halfpi = stat.tile([128, 1], F32, tag="halfpi")
        nc.gpsimd.memset(halfpi[:], HALF_PI)
        nc.const_aps.aps[(F32, HALF_PI)] = halfpi[:]

        # ---------- t=0: hoist the Sin table load with a dummy ----------
        sin_dummy = stat.tile([128, 1], F16, tag="sin_dummy")
        nc.scalar.activation(sin_dummy[:], halfpi[:], AF.Sin, bias=0.0,
                             scale=1.0)

        # exp bias constant (per-partition AP)
        neg6 = stat.tile([128, 1], F32, tag="neg6")
        nc.gpsimd.memset(neg6[:], EXP_SHIFT)

        # ---------- t=0: PE prewarm (p-state ramp covers the DMA wait).
        # Dummy fp16 matmuls [1,32]; ~46 back-to-back ~= 3us of PE busy.
        warm_a = const.tile([128, 1], F16, tag="warm_a")
        nc.gpsimd.memset(warm_a[:], 0.0)
        warm_b = const.tile([128, 32], F16, tag="warm_b")
        nc.gpsimd.memset(warm_b[:], 0.0)
        warm_bank = ps_sm.tile([128, 512], F32, tag="sm_bank")
        warm_ps = warm_bank[0:1, 384:416]
        for i in range(10):
            nc.tensor.matmul(warm_ps, lhsT=warm_a[:], rhs=warm_b[:],
                             start=True, stop=True)

        # ---------- DMAs (SP queue, order = dependency priority) ----------
        # q path leads: its Sins gate the F tiles that gate every score matmul
        w_sb = const.tile([E, 2 * D], F32, tag="w_sb")
        nc.sync.dma_start(w_sb[:], w_d[:])
        vt_sb = const.tile([E, 1], F32, tag="vt_sb")
        nc.gpsimd.dma_start(vt_sb[:], vt_d[:])
        qplain = const.tile([128, QSH], F32, tag="qplain")
        nc.sync.dma_start(qplain[:].rearrange("p (t j) -> p t j", j=128),
                          q_d[:].rearrange("(t p) j -> p t j", p=128))
        kplain = const.tile([128, LK], F32, tag="kplain")
        for h in range(2):
            nc.sync.dma_start(
                kplain[:, h * 512:(h + 1) * 512].rearrange("p (t j) -> p t j", j=128),
                k_d[h * 512:(h + 1) * 512, :].rearrange("(t p) j -> p t j", p=128))
        vplain = const.tile([128, LK], F32, tag="vplain")
        nc.sync.dma_start(vplain[:].rearrange("p (t j) -> p t j", j=128),
                          v_d[:].rearrange("(t p) j -> p t j", p=128))

        # ---------- constants / coefficient vectors ----------
        ident = const.tile([128, 128], F32, tag="ident")
        make_identity(nc, ident[:])

        def coef_vec(name, scale):
            v = stat.tile([E, 1], F32, tag=name)
            with tc.high_priority():
                nc.gpsimd.tensor_scalar(v[:], vt_sb[:], float(scale), None,
                                        op0=ALU.mult)
            return v

        v_b1 = coef_vec("v_b1", B1)
        v_2b2 = coef_vec("v_2b2", 2 * B2)
        v_m4b2 = coef_vec("v_m4b2", -4 * B2)
        v_3b3 = coef_vec("v_3b3", 3 * B3)
        v_b3 = coef_vec("v_b3", B3)
        v_m4b3 = coef_vec("v_m4b3", -4 * B3)
        v_4b4 = coef_vec("v_4b4", 4 * B4)
        v_m32b4 = coef_vec("v_m32b4", -32 * B4)
        cvT = stat.tile([E, 1], F32, tag="cvT")
        nc.gpsimd.tensor_scalar(cvT[:], vt_sb[:], C_LIN, None, op0=ALU.mult)

        # ---------- transposes: W (raw; angle scale folded into Sin), q, k ----------
        wT = const.tile([D, 2 * E], F32R, tag="wT")     # [wqT | wkT]
        pw = ps_set.tile([128, 512], F32, tag="pset")
        nc.tensor.transpose(pw[:, 0:128], w_sb[:, 0:D], ident[:])
        nc.tensor.transpose(pw[:, 128:256], w_sb[:, D:2 * D], ident[:])
        nc.vector.tensor_copy(wT[:], pw[:, 0:256])

        queryT = const.tile([D, QSH], F32R, tag="queryT")
        pq = ps_set.tile([128, 512], F32, tag="pset")
        for g in range(2):
            nc.tensor.transpose(pq[:, g * 128:(g + 1) * 128],
                                qplain[:, g * 128:(g + 1) * 128], ident[:])
        nc.vector.tensor_copy(queryT[:], pq[:, 0:256])

        # base_q + q sins first (high priority: they gate all F tiles)
        sm_bank = warm_bank
        base_q = sm_bank[:, 0:QSH]
        with tc.high_priority():
            nc.tensor.matmul(base_q, lhsT=wT[:, 0:128], rhs=queryT[:],
                             start=True, stop=True)
            s1q = qprod.tile([E, QSH], F16, tag="s1q")
            nc.scalar.activation(s1q[:], base_q, AF.Sin, scale=ANG)
            c1q = qprod.tile([E, QSH], F16, tag="c1q")
            nc.scalar.activation(c1q[:], base_q, AF.Sin, bias=HALF_PI,
                                 scale=ANG)

        keyT = const.tile([D, LK], F32R, tag="keyT")
        for h in range(2):
            pk = ps_set.tile([128, 512], F32, tag="pset")
            for t in range(4):
                j = h * 4 + t
                nc.tensor.transpose(pk[:, t * 128:(t + 1) * 128],
                                    kplain[:, j * 128:(j + 1) * 128], ident[:])
            if h == 0:
                with tc.high_priority():
                    nc.vector.tensor_copy(keyT[:, 0:512], pk[:])
            else:
                nc.vector.tensor_copy(keyT[:, 512:1024], pk[:])

        # linear-term bias, pushed through the projection:
        # wkvt[d] = sum_e Wk[e,d] * C_LIN*vT[e]; bias_col[k] = keyT^T @ wkvt
        with tc.high_priority():
            wkvt_ps = sm_bank[:, 256:257]
            nc.tensor.matmul(wkvt_ps, lhsT=w_sb[:, D:2 * D], rhs=cvT[:],
                             start=True, stop=True)
            wkvt = stat.tile([128, 1], F32, tag="wkvt")
            nc.vector.tensor_copy(wkvt[:], wkvt_ps)
            wps = sm_bank[:, 264:264 + NKT]
            for j in range(NKT):
                nc.tensor.matmul(wps[:, j:j + 1],
                                 lhsT=keyT[:, j * 128:(j + 1) * 128].bitcast(F32),
                                 rhs=wkvt[:],
                                 start=True, stop=True)

        # ---------- k-side features ----------
        # Act: s1,c1; DVE/Pool: products.  Per-chunk G tiles (blk0 in two
        # 256-wide chunks, blk1 one 512) so early score matmuls never wait
        # on later chunks' writers.
        GNAMES = ('s1', 'c1', 'P1', 'C2', 's3', 'c3', 'P2', 'C4')
        CHUNKS = [(0, 0, 256), (1, 256, 256), (2, 512, 512)]  # (idx, k0, width)
        G = {}   # (chunk, name) -> [E, width] f16 tile

        def k_chunk(c, bk_part, width):
            for nm in GNAMES:
                G[(c, nm)] = kG.tile([E, width], F16, tag=f"G{c}_{nm}",
                                     name=f"G{c}_{nm}")
            s1, c1 = G[(c, 's1')], G[(c, 'c1')]
            nc.scalar.activation(s1[:], bk_part, AF.Sin, scale=ANG)
            nc.scalar.activation(c1[:], bk_part, AF.Sin, bias=HALF_PI,
                                 scale=ANG)
            qa = kraw.tile([E, width], F16, tag="qa", bufs=3)
            nc.gpsimd.tensor_mul(qa[:], s1[:], s1[:])
            nc.gpsimd.tensor_mul(G[(c, 'P1')][:], s1[:], c1[:])
            nc.vector.tensor_scalar(G[(c, 'C2')][:], qa[:], -2.0, 1.0,
                                    op0=ALU.mult, op1=ALU.add)
            t3 = kraw.tile([E, width], F16, tag="t3", bufs=3)
            nc.vector.tensor_scalar(t3[:], qa[:], -4.0, 3.0,
                                    op0=ALU.mult, op1=ALU.add)
            nc.vector.tensor_mul(G[(c, 's3')][:], s1[:], t3[:])
            t3b = kraw.tile([E, width], F16, tag="t3b", bufs=3)
            nc.vector.tensor_scalar(t3b[:], qa[:], -4.0, 1.0,
                                    op0=ALU.mult, op1=ALU.add)
            nc.vector.tensor_mul(G[(c, 'c3')][:], c1[:], t3b[:])
            nc.gpsimd.tensor_mul(G[(c, 'P2')][:], G[(c, 'P1')][:],
                                 G[(c, 'C2')][:])
            qb = kraw.tile([E, width], F16, tag="qb", bufs=3)
            nc.vector.tensor_mul(qb[:], G[(c, 'P1')][:], G[(c, 'P1')][:])
            nc.vector.tensor_scalar(G[(c, 'C4')][:], qb[:], -8.0, 1.0,
                                    op0=ALU.mult, op1=ALU.add)

        def g_slice(name, j):
            """lhsT slice for k-tile j (128 cols)"""
            if j < 2:
                c, off = 0, j * 128
            elif j < 4:
                c, off = 1, (j - 2) * 128
            else:
                c, off = 2, (j - 4) * 128
            return G[(c, name)][:, off:off + 128]

        base_k = [None, None]
        for h in range(NBLK):
            bk = ps_set.tile([128, 512], F32, tag="pset")
            base_k[h] = bk
            nc.tensor.matmul(bk[:], lhsT=wT[:, 128:256],
                             rhs=keyT[:, h * 512:(h + 1) * 512],
                             start=True, stop=True)
            if h == 0:
                k_chunk(0, bk[:, 0:256], 256)
                k_chunk(1, bk[:, 256:512], 256)
            else:
                k_chunk(2, bk[:], 512)

        # ---------- q-side products + coefficient-folded F tiles ----------
        # pairs: (F1s,c1k) (F1c,s1k) (F2s,C2k) (F2c,P1k)
        #        (F3s,c3k) (F3c,s3k) (F4s,C4k) (F4c,P2k)
        _hp = tc.high_priority()
        _hp.__enter__()
        F1c = qprod.tile([E, QSH], F16, tag="F1c")
        nc.vector.tensor_scalar(F1c[:], c1q[:], v_b1[:], None, op0=ALU.mult)
        F1s = qprod.tile([E, QSH], F16, tag="F1s")
        nc.vector.tensor_scalar(F1s[:], s1q[:], v_b1[:], None, op0=ALU.mult)
        qa_q = qprod.tile([E, QSH], F16, tag="qa_q")
        nc.vector.tensor_mul(qa_q[:], s1q[:], s1q[:])
        P1_q = qprod.tile([E, QSH], F16, tag="P1_q")
        nc.gpsimd.tensor_mul(P1_q[:], s1q[:], c1q[:])
        C2_q = qprod.tile([E, QSH], F16, tag="C2_q")
        nc.vector.tensor_scalar(C2_q[:], qa_q[:], -2.0, 1.0,
                                op0=ALU.mult, op1=ALU.add)
        qb_q = qprod.tile([E, QSH], F16, tag="qb_q")
        nc.gpsimd.tensor_mul(qb_q[:], P1_q[:], P1_q[:])

        F2s = qprod.tile([E, QSH], F16, tag="F2s")
        nc.vector.tensor_scalar(F2s[:], P1_q[:], v_2b2[:], None, op0=ALU.mult)
        F2c = qprod.tile([E, QSH], F16, tag="F2c")
        nc.vector.tensor_scalar(F2c[:], qa_q[:], v_m4b2[:], v_2b2[:],
                                op0=ALU.mult, op1=ALU.add)
        t3q = qprod.tile([E, QSH], F16, tag="t3q")
        nc.vector.tensor_scalar(t3q[:], qa_q[:], v_m4b3[:], v_3b3[:],
                                op0=ALU.mult, op1=ALU.add)
        F3s = qprod.tile([E, QSH], F16, tag="F3s")
        nc.vector.tensor_mul(F3s[:], s1q[:], t3q[:])
        t3bq = qprod.tile([E, QSH], F16, tag="t3bq")
        nc.vector.tensor_scalar(t3bq[:], qa_q[:], v_m4b3[:], v_b3[:],
                                op0=ALU.mult, op1=ALU.add)
        F3c = qprod.tile([E, QSH], F16, tag="F3c")
        nc.vector.tensor_mul(F3c[:], c1q[:], t3bq[:])
        t4q = qprod.tile([E, QSH], F16, tag="t4q")
        nc.vector.tensor_scalar(t4q[:], P1_q[:], v_4b4[:], None, op0=ALU.mult)
        F4s = qprod.tile([E, QSH], F16, tag="F4s")
        nc.vector.tensor_mul(F4s[:], t4q[:], C2_q[:])
        F4c = qprod.tile([E, QSH], F16, tag="F4c")
        nc.vector.tensor_scalar(F4c[:], qb_q[:], v_m32b4[:], v_4b4[:],
                                op0=ALU.mult, op1=ALU.add)

        _hp.__exit__(None, None, None)
        # ordered by k-side tile readiness: s1 (Act), C2 (ts of qa), c1
        # (Act), s3, P1, c3, P2, C4 -- keeps the PE score stream stall-free
        pairs = [(F1c, 's1'), (F2s, 'C2'), (F1s, 'c1'), (F3c, 's3'),
                 (F2c, 'P1'), (F3s, 'c3'), (F4c, 'P2'), (F4s, 'C4')]

        # ---------- scores (transposed): per k-tile-pair PSUM bank ----------
        # bank jj holds k-tiles (2jj, 2jj+1): [128k, 2*256q]
        score_ps = []
        for jj in range(NKT // 2):
            sc = ps_sc.tile([128, 512], F32, tag="sc")
            score_ps.append(sc)
            for t2 in range(2):
                j = 2 * jj + t2
                for ci, (f, gname) in enumerate(pairs):
                    nc.tensor.matmul(sc[:, t2 * 256:(t2 + 1) * 256],
                                     lhsT=g_slice(gname, j), rhs=f[:],
                                     start=(ci == 0), stop=(ci == 7))

        # ---------- value scaling by exp(bias): Exp phase on Act ----------
        # gate: zero-valued bias that data-depends on the LAST Sin output, so
        # the scheduler cannot hoist this Exp between Sins (each Sin<->Exp
        # switch costs a 1283ns activation-table reload).
        gate = stat.tile([128, 1], F32, tag="gate")
        nc.vector.tensor_scalar(gate[:], G[(2, 'c3')][0:128, 511:512], 0.0, None,
                                op0=ALU.mult)
        wcol = stat.tile([128, NKT], F32, tag="wcol")
        nc.scalar.activation(wcol[:], wps, AF.Exp, bias=gate[:], scale=1.0)
        val16 = const.tile([128, NKT * 129], F16, tag="val16")
        for j in range(NKT):
            eng = nc.vector if j % 2 == 0 else nc.gpsimd
            eng.tensor_scalar(val16[:, j * 129:j * 129 + 128],
                              vplain[:, j * 128:(j + 1) * 128],
                              wcol[:, j:j + 1], None, op0=ALU.mult)
        # denominator column = w itself (strided copy into col 128 of each)
        vcols = val16[:].rearrange("p (t j) -> p t j", j=129)
        nc.vector.tensor_copy(vcols[:, :, 128], wcol[:])

        # ---------- softmax exp + AV per bank ----------
        pav_t = [ps_sc.tile([128, 512], F32, tag="sc", name=f"pavb{g}")
                 for g in range(2)]
        pav = [pav_t[0][:, 0:129], pav_t[1][:, 0:129]]
        for jj in range(NKT // 2):
            p = probs_p.tile([128, 512], F16, tag="P")
            nc.scalar.activation(p[:], score_ps[jj][:], AF.Exp, bias=neg6[:])
            for t2 in range(2):
                j = 2 * jj + t2
                for g in range(2):
                    nc.tensor.matmul(pav[g],
                                     lhsT=p[:, t2 * 256 + g * 128:
                                            t2 * 256 + (g + 1) * 128],
                                     rhs=val16[:, j * 129:(j + 1) * 129],
                                     start=(j == 0), stop=(j == NKT - 1))

        # ---------- normalize + output ----------
        osb = outp.tile([128, QSH], F32, tag="osb")
        for g in range(2):
            rinv = stat.tile([128, 1], F32, tag="rinv")
            nc.vector.reciprocal(rinv[:], pav[g][:, 128:129])
            nc.vector.tensor_scalar(osb[:, g * 128:(g + 1) * 128],
                                    pav[g][:, 0:128], rinv[:], None,
                                    op0=ALU.mult)
            nc.sync.dma_start(out_d[g * 128:(g + 1) * 128, :],
                              osb[:, g * 128:(g + 1) * 128])


def _drop_trailing_range_clear(nc):
    """This walrus rejects the raw EVENT_SEMAPHORE_RANGE_CLEAR InstISA
    ("ISA wrong length").  Tile emits exactly one, at the kernel tail, to
    recycle pool semaphores for later tiles — of which there are none, so
    dropping it is safe.  Verified: no later instruction waits on the range."""
    import re
    for f in nc.m.functions:
        for blk in f.blocks:
            insts = list(blk.instructions)
            keep, pending = [], []
            for ins in insts:
                if (type(ins).__name__ == "InstISA"
                        and "EVENT_SEMAPHORE_RANGE_CLEAR" in ins.concise()):
                    m = re.search(r"range_first=(\d+) range_last=(\d+)", ins.concise())
                    pending.append((ins, set(range(int(m.group(1)), int(m.group(2)) + 1))))
                    continue
                for _, rng in pending:
                    si = ins.sync_info
                    if si is not None:
                        used = {w.id for w in si.on_wait} | {u.id for u in si.on_update}
                        assert not (used & rng), (
                            f"range-clear removal unsafe: {ins.name} uses {used & rng}")
                keep.append(ins)
            blk.instructions = keep


def split_excess_waits(nc, max_waits=1):
    """This walrus rejects >1 sync-wait per instruction; move extras onto
    preceding no-ops on the same engine (engines issue in order, so a wait
    on an earlier instruction subsumes one on the original)."""
    _drop_trailing_range_clear(nc)
    n = 0
    for f in nc.m.functions:
        for blk in f.blocks:
            new_list = []
            for ins in blk.instructions:
                si = ins.sync_info
                if si is not None and len(si.on_wait) > max_waits:
                    waits = list(si.on_wait)
                    extra, keep = waits[:-max_waits], waits[-max_waits:]
                    for j in range(0, len(extra), max_waits):
                        nop = mybir.InstNoOp(
                            name=f"{ins.name}-ws{j}",
                            engine=ins.engine,
                            sync_info=mybir.SyncInfo(on_wait=extra[j:j + max_waits],
                                                     on_update=[]),
                            bass_nofuse=True,
                        )
                        new_list.append(nop)
                    ins.sync_info = mybir.SyncInfo(on_wait=keep,
                                                  on_update=list(si.on_update))
                    n += 1
                new_list.append(ins)
            blk.instructions = new_list
    return n


_CACHED_NC = None


def _get_nc():
    global _CACHED_NC
    if _CACHED_NC is None:
        nc = build_nc()
        split_excess_waits(nc)
        _CACHED_NC = nc
    return _CACHED_NC


def make_in_maps(query, key, value, vT, weight):
    query = np.ascontiguousarray(np.asarray(query, np.float32))
    key = np.ascontiguousarray(np.asarray(key, np.float32))
    value = np.ascontiguousarray(np.asarray(value, np.float32))
    vT = np.ascontiguousarray(np.asarray(vT, np.float32)).reshape(E, 1)
    weight = np.ascontiguousarray(np.asarray(weight, np.float32))
    in_maps = []
    for c in range(N_CORES):
        b, qs = divmod(c, N_CORES // B)
        in_maps.append({
            "q": np.ascontiguousarray(query[b, qs * QSH:(qs + 1) * QSH]),
            "k": key[b],
            "v": value[b],
            "w": weight,
            "vt": vT,
        })
    return in_maps


def kernel(query, key, value, vT, weight):
    nc = _get_nc()
    in_maps = make_in_maps(query, key, value, vT, weight)
    res = run_bass_kernel_spmd(nc, in_maps, core_ids=list(range(N_CORES)))
    out = np.empty((B, LQ, VD), np.float32)
    for c in range(N_CORES):
        b, qs = divmod(c, N_CORES // B)
        out[b, qs * QSH:(qs + 1) * QSH] = res.results[c]["out"]
    return out



# revision 29
# speedup vs baseline: 1.4686x; 1.4686x over previous
"""Additive (Bahdanau) attention on 8 Trainium2 cores.

Math: scores[b,q,k] = sum_e vT[e] * tanh(qp[b,q,e] + kp[b,k,e]);
out = softmax_k(scores) @ value.  qp = query @ Wq^T, kp = key @ Wk^T.

tanh(z) ~ c*z + sum_{m=1..3} b_m sin(m*pi*z/L) on the data range.  The
angle-addition identity factorizes each harmonic into separable q/k
products -> TensorE matmuls contracting over E.  Harmonics 2,3 derive
from the single sin/cos pair per side (2 ScalarE Sins per side) via
double/triple-angle identities on DVE/Pool:
    sin2 = 2 s c, cos2 = 1-2s^2, sin3 = s(3-4s^2), cos3 = c(1-4s^2).
The linear term's q-part is softmax-invariant (dropped); its k-part is
exact via exp(bias_k)-scaled value rows, with the denominator obtained
from a 129th all-w column in the AV matmul.  exp(bias_k) is evaluated as
a degree-6 polynomial on Pool (bias_k in [-0.7, 0.7]) so the Activation
engine pays only ONE Sin->Exp table switch, hoisted behind a dummy exp
whose input aliases the last Sin's output column (ordering by data dep).

Host-side prep (pure layout, in make_in_maps): q/k are transposed so the
feature dim lands on partitions, v is tile-interleaved, and Wq^T plus all
vT-derived coefficient vectors ride in one packed tensor.  This removes
every PE transpose and PSUM->SBUF staging copy from the device kernel.

Scores are built TRANSPOSED ([k-tile, q]) so softmax probabilities come
out of the Exp activation already in the layout the AV matmul needs.
The score matmul stream is pair-progressive over the first four k-tiles
(all tiles' pair p before pair p+1) because the late G products (s3/c3)
only exist ~1.7us after the block sins; the second four k-tiles run
tile-major so each PSUM bank closes as early as possible for its Exp.

Sharding: core = (batch, q-block): 2 batches x 4 q-blocks of 256 rows.
Each core computes its full attention independently; no collectives.
"""

import numpy as np

import concourse.bass as bass
import concourse.tile as tile
from concourse import mybir
from concourse.bass_utils import run_bass_kernel_spmd

F32 = mybir.dt.float32
F32R = mybir.dt.float32r
F16 = mybir.dt.float16
AF = mybir.ActivationFunctionType
ALU = mybir.AluOpType

# ---- problem shapes (hardcoded per contract) ----
B, LQ, LK, D, E, VD = 2, 1024, 1024, 128, 128, 128
N_CORES = 8
QSH = (B * LQ) // N_CORES          # 256 q rows per core
NKT = LK // 128                    # 8 k-tiles of 128

# ---- tanh approximation: c*z + sum_{m=1..3} b_m sin(m*pi*z/L), fit on
# the data distribution (bulk sigma~1.18, |z|<8.7 guard) ----
L_PER = 4.7
C_LIN = 0.19558908
B1 = 0.59605625
B2 = 0.12020409
B3 = 0.09591005
ANG = float(np.float32(np.pi / L_PER))     # z -> sin angle
HALF_PI = float(np.float32(np.pi / 2))
EXP_SHIFT = -6.0

# exp(x) on [-0.7, 0.7] as a poly (for the linear-term bias); Horner order
EXP_POLY = [0.00140656, 0.00848392, 0.04166343, 0.1666346,
            0.50000006, 1.00000166, 1.0]

# wpack column layout: [wqT | wkT | coef columns | wkvt]
COL_B1, COL_2B2, COL_M4B2, COL_3B3, COL_B3, COL_M4B3, COL_WKVT = range(256, 263)
WQP_W = 263


def build_nc():
    nc = bass.Bass("TRN2", target_bir_lowering=False, debug=False)

    wqp_d = nc.dram_tensor("wqp", [128, WQP_W], F16, kind="ExternalInput").ap()
    qT_d = nc.dram_tensor("qT", [D, QSH], F16, kind="ExternalInput").ap()
    kT_d = nc.dram_tensor("kT", [D, LK], F16, kind="ExternalInput").ap()
    v_d = nc.dram_tensor("v", [128, LK], F16, kind="ExternalInput").ap()
    out_d = nc.dram_tensor("out", [128, QSH], F16, kind="ExternalOutput").ap()

    with tile.TileContext(nc) as tc:
        _body(tc, wqp_d, qT_d, kT_d, v_d, out_d)
    return nc


def _body(tc, wqp_d, qT_d, kT_d, v_d, out_d):
    nc = tc.nc
    from contextlib import ExitStack
    ctx = ExitStack()
    with ctx:
        const = ctx.enter_context(tc.tile_pool(name="const", bufs=1))
        kG = ctx.enter_context(tc.tile_pool(name="kG", bufs=1))
        kraw = ctx.enter_context(tc.tile_pool(name="kraw", bufs=2))
        qprod = ctx.enter_context(tc.tile_pool(name="qprod", bufs=1))
        probs_p = ctx.enter_context(tc.tile_pool(name="probs", bufs=5))
        outp = ctx.enter_context(tc.tile_pool(name="outp", bufs=1))
        stat = ctx.enter_context(tc.tile_pool(name="stat", bufs=2))
        ps_sc = ctx.enter_context(tc.tile_pool(name="ps_sc", bufs=4, space="PSUM"))
        ps_bk = ctx.enter_context(tc.tile_pool(name="ps_bk", bufs=2, space="PSUM"))
        ps_sm = ctx.enter_context(tc.tile_pool(name="ps_sm", bufs=1, space="PSUM"))

        # Sin's float bias must be a pre-registered const AP (sundagen only
        # accepts immediate bias for Copy/Reciprocal); tile-tracked so the
        # memset->read dependency is synced without an all-engine barrier.
        halfpi = stat.tile([128, 1], F32, tag="halfpi")
        nc.gpsimd.memset(halfpi[:], HALF_PI)
        nc.const_aps.aps[(F32, HALF_PI)] = halfpi[:]

        # ---------- t~0: hoist the Sin table load with a dummy.  Emitted
        # before ACT's kT dma_starts so its ENGINE work overlaps their SEQ
        # issue instead of queueing behind them. ----------
        sin_dummy = stat.tile([128, 1], F16, tag="sin_dummy")
        nc.scalar.activation(sin_dummy[:], halfpi[:], AF.Sin, bias=0.0,
                             scale=1.0)

        # ---------- DMAs ----------
        # Transfers serialize globally on the DMA engines, so ARRIVAL ORDER
        # at that resource is what matters; the SWDGE (Pool) queue's sem
        # propagation is ~900ns slower than HWDGE, so everything rides the
        # two HWDGE queues.  SP: qT (longest dependent chain), wpack, kT1,
        # v halves.  ACT: kT0 only -- its SEQ issue overlaps the dummy's
        # engine time, and a second ACT DMA would delay s1q's dispatch.
        qT = const.tile([D, QSH], F16, tag="qT")
        nc.sync.dma_start(qT[:], qT_d[:])
        wqp = const.tile([128, WQP_W], F16, tag="wqp")
        nc.gpsimd.dma_start(wqp[:], wqp_d[:])
        kT = const.tile([D, LK], F16, tag="kT")
        nc.scalar.dma_start(kT[:, 0:512], kT_d[:, 0:512])
        nc.sync.dma_start(kT[:, 512:1024], kT_d[:, 512:1024])
        vplain = const.tile([128, LK], F16, tag="vplain")
        nc.sync.dma_start(vplain[:, 0:512], v_d[:, 0:512])
        nc.sync.dma_start(vplain[:, 512:1024], v_d[:, 512:1024])

        # exp bias constant (per-partition AP)
        neg6 = stat.tile([128, 1], F32, tag="neg6")
        nc.gpsimd.memset(neg6[:], EXP_SHIFT)

        # ---------- t~0: PE prewarm pins pe_busy_start early ----------
        warm_a = const.tile([128, 1], F16, tag="warm_a")
        nc.gpsimd.memset(warm_a[:], 0.0)
        warm_b = const.tile([128, 32], F16, tag="warm_b")
        nc.gpsimd.memset(warm_b[:], 0.0)
        sm_bank = ps_sm.tile([128, 512], F32, tag="sm_bank")

        osb = outp.tile([128, QSH], F16, tag="osb")
        warm_ps = sm_bank[0:1, 384:416]
        for i in range(10):
            nc.tensor.matmul(warm_ps, lhsT=warm_a[:], rhs=warm_b[:],
                             start=True, stop=True)

        # tensor_scalar requires f32 scalar APs; the f16 coef columns get
        # one tiny upconvert copy (Pool, off the critical path).
        coefs32 = stat.tile([128, 7], F32, tag="coefs32")
        nc.gpsimd.tensor_copy(coefs32[:], wqp[:, 256:263])

        # ---------- q chain: base_q -> s1q/c1q ----------
        base_q = sm_bank[:, 0:QSH]
        with tc.high_priority():
            nc.tensor.matmul(base_q, lhsT=wqp[:, 0:128], rhs=qT[:],
                             start=True, stop=True)
            s1q = qprod.tile([E, QSH], F16, tag="s1q")
            nc.scalar.activation(s1q[:], base_q, AF.Sin, scale=ANG)
            c1q = qprod.tile([E, QSH], F16, tag="c1q")
            nc.scalar.activation(c1q[:], base_q, AF.Sin, bias=HALF_PI,
                                 scale=ANG)

        # ---------- k-side sins + products ----------
        GNAMES = ('s1', 'c1', 'P1', 'C2', 's3', 'c3')
        G = {}
        for h in range(2):
            for nm in GNAMES:
                G[(h, nm)] = kG.tile([E, 512], F16, tag=f"G{h}_{nm}",
                                     name=f"G{h}_{nm}")

        base_k = []
        for h in range(2):
            bk = ps_bk.tile([128, 512], F32, tag="pbk")
            base_k.append(bk)
            nc.tensor.matmul(bk[:], lhsT=wqp[:, 128:256],
                             rhs=kT[:, h * 512:(h + 1) * 512],
                             start=True, stop=True)

        def k_sins(h):
            nc.scalar.activation(G[(h, 's1')][:], base_k[h][:], AF.Sin,
                                 scale=ANG)
            nc.scalar.activation(G[(h, 'c1')][:], base_k[h][:], AF.Sin,
                                 bias=HALF_PI, scale=ANG)

        def k_products(h):
            s1, c1 = G[(h, 's1')], G[(h, 'c1')]
            # Pool: qa, t3b, c3;  DVE: P1, C2, t3, s3
            qa = kraw.tile([E, 512], F16, tag="qa")
            nc.gpsimd.tensor_mul(qa[:], s1[:], s1[:])
            t3b = kraw.tile([E, 512], F16, tag="t3b")
            nc.gpsimd.tensor_scalar(t3b[:], qa[:], -4.0, 1.0,
                                    op0=ALU.mult, op1=ALU.add)
            nc.gpsimd.tensor_mul(G[(h, 'c3')][:], c1[:], t3b[:])
            nc.vector.tensor_mul(G[(h, 'P1')][:], s1[:], c1[:])
            nc.vector.tensor_scalar(G[(h, 'C2')][:], qa[:], -2.0, 1.0,
                                    op0=ALU.mult, op1=ALU.add)
            t3 = kraw.tile([E, 512], F16, tag="t3")
            nc.vector.tensor_scalar(t3[:], qa[:], -4.0, 3.0,
                                    op0=ALU.mult, op1=ALU.add)
            nc.vector.tensor_mul(G[(h, 's3')][:], s1[:], t3[:])

        def g_slice(name, j):
            h, off = divmod(j, 4)
            return G[(h, name)][:, off * 128:off * 128 + 128]

        k_sins(0)
        k_products(0)
        k_sins(1)

        # ---------- q-side products + coefficient-folded F tiles ----------
        _hp = tc.high_priority()
        _hp.__enter__()
        F1s = qprod.tile([E, QSH], F16, tag="F1s")
        nc.vector.tensor_scalar(F1s[:], s1q[:], coefs32[:, COL_B1 - 256:COL_B1 - 255],
                                None, op0=ALU.mult)
        qa_q = qprod.tile([E, QSH], F16, tag="qa_q")
        nc.vector.tensor_mul(qa_q[:], s1q[:], s1q[:])
        F1c = qprod.tile([E, QSH], F16, tag="F1c")
        nc.vector.tensor_scalar(F1c[:], c1q[:], coefs32[:, COL_B1 - 256:COL_B1 - 255],
                                None, op0=ALU.mult)
        F2c = qprod.tile([E, QSH], F16, tag="F2c")
        nc.vector.tensor_scalar(F2c[:], qa_q[:], coefs32[:, COL_M4B2 - 256:COL_M4B2 - 255],
                                coefs32[:, COL_2B2 - 256:COL_2B2 - 255],
                                op0=ALU.mult, op1=ALU.add)
        P1_q = qprod.tile([E, QSH], F16, tag="P1_q")
        nc.gpsimd.tensor_mul(P1_q[:], s1q[:], c1q[:])
        t3q = qprod.tile([E, QSH], F16, tag="t3q")
        nc.vector.tensor_scalar(t3q[:], qa_q[:], coefs32[:, COL_M4B3 - 256:COL_M4B3 - 255],
                                coefs32[:, COL_3B3 - 256:COL_3B3 - 255],
                                op0=ALU.mult, op1=ALU.add)
        F3s = qprod.tile([E, QSH], F16, tag="F3s")
        nc.vector.tensor_mul(F3s[:], s1q[:], t3q[:])
        t3bq = qprod.tile([E, QSH], F16, tag="t3bq")
        nc.vector.tensor_scalar(t3bq[:], qa_q[:], coefs32[:, COL_M4B3 - 256:COL_M4B3 - 255],
                                coefs32[:, COL_B3 - 256:COL_B3 - 255],
                                op0=ALU.mult, op1=ALU.add)
        F3c = qprod.tile([E, QSH], F16, tag="F3c")
        nc.vector.tensor_mul(F3c[:], c1q[:], t3bq[:])
        F2s = qprod.tile([E, QSH], F16, tag="F2s")
        nc.vector.tensor_scalar(F2s[:], P1_q[:], coefs32[:, COL_2B2 - 256:COL_2B2 - 255],
                                None, op0=ALU.mult)
        _hp.__exit__(None, None, None)

        k_products(1)

        # ordered by k-side tile readiness: s1, c1 (Act), C2, P1, s3, c3
        pairs = [(F1c, 's1'), (F1s, 'c1'), (F2s, 'C2'), (F2c, 'P1'),
                 (F3c, 's3'), (F3s, 'c3')]

        # linear-term bias columns: wps[:, j] = kT_tile^T @ wkvt
        wps = sm_bank[:, 264:264 + NKT]

        def emit_wps(js):
            for j in js:
                nc.tensor.matmul(wps[:, j:j + 1],
                                 lhsT=kT[:, j * 128:(j + 1) * 128],
                                 rhs=wqp[:, COL_WKVT:COL_WKVT + 1],
                                 start=True, stop=True)

        emit_wps(range(0, 4))
        emit_wps(range(4, 8))

        # ---------- linear-term exp via Pool polynomial ----------
        wcol = stat.tile([128, NKT], F32, tag="wcol")
        acc = stat.tile([128, NKT], F32, tag="expacc")
        nc.vector.tensor_scalar(acc[:], wps, EXP_POLY[0], EXP_POLY[1],
                                op0=ALU.mult, op1=ALU.add)
        for cpoly in EXP_POLY[2:-1]:
            nc.vector.scalar_tensor_tensor(acc[:], acc[:], float(cpoly), wps,
                                           op0=ALU.bypass, op1=ALU.mult)
            nc.vector.tensor_scalar(acc[:], acc[:], 1.0, cpoly,
                                    op0=ALU.mult, op1=ALU.add)
        nc.vector.scalar_tensor_tensor(acc[:], acc[:], 1.0, wps,
                                       op0=ALU.mult, op1=ALU.mult)
        nc.vector.tensor_scalar(wcol[:], acc[:], 1.0, EXP_POLY[-1],
                                op0=ALU.mult, op1=ALU.add)

        val16 = const.tile([128, NKT * 129], F16, tag="val16")
        for j in range(NKT):
            eng = nc.vector if j % 2 == 0 else nc.gpsimd
            eng.tensor_scalar(val16[:, j * 129:j * 129 + 128],
                              vplain[:, j * 128:(j + 1) * 128],
                              wcol[:, j:j + 1], None, op0=ALU.mult)
        vcols = val16[:].rearrange("p (t j) -> p t j", j=129)
        nc.vector.tensor_copy(vcols[:, :, 128], wcol[:])

        # ---------- scores (transposed) ----------
        # One OPEN accumulation group per 2KB PSUM bank, and a bank's groups
        # must be strictly sequential.  Banks pair (t_i, t_{i+4}) so ALL
        # FOUR early tiles sit in distinct banks and can advance
        # pair-progressively while the late G products (P1, s3, c3) mature;
        # tiles 4-7 then run tile-major so each bank closes early for its
        # Exp.  (t3, t7) share the last bank: t3 closes first, so its Exp
        # can fire the moment the table reload finishes.
        sc_banks = [ps_sc.tile([128, 512], F32, tag="sc", name=f"scb{i}")
                    for i in range(4)]
        sc_regions = {}
        for i in range(4):
            sc_regions[i] = sc_banks[i][:, 0:256]
            sc_regions[i + 4] = sc_banks[i][:, 256:512]

        def emit_tile(j, cis):
            for ci in cis:
                f, gname = pairs[ci]
                nc.tensor.matmul(sc_regions[j], lhsT=g_slice(gname, j),
                                 rhs=f[:], start=(ci == 0), stop=(ci == 5))

        for ci in range(6):
            for j in range(4):
                emit_tile(j, [ci])
        for j in range(4, 8):
            emit_tile(j, range(6))

        # ---------- exp table switch: dummy reads the last Sin's output ----------
        exp_dummy = stat.tile([128, 1], F16, tag="exp_dummy")
        nc.scalar.activation(exp_dummy[:], G[(1, 'c1')][0:128, 511:512],
                             AF.Exp, bias=neg6[:])

        # ---------- softmax exp + AV ----------
        # One 612-wide Exp per bank as it closes (tile-granularity dep
        # tracking forbids reading a bank's early half before its late
        # half is written).  Two concurrently-open PSUM accumulation
        # groups must live in different banks: g0 reuses sm_bank (its
        # earlier groups are closed by now), g1 recycles base_k bank 0
        # (dead once the G0 sins read it).
        pav1_t = ps_bk.tile([128, 512], F32, tag="pbk", name="pav1")
        pav = [sm_bank[:, 0:129], pav1_t[:, 0:129]]
        for bi in range(4):
            p = probs_p.tile([128, 512], F16, tag="P")
            nc.scalar.activation(p[:], sc_banks[bi][:], AF.Exp, bias=neg6[:])
            for j in (bi, bi + 4):
                for g in range(2):
                    nc.tensor.matmul(pav[g],
                                     lhsT=p[:, (0 if j < 4 else 256) + g * 128:
                                            (0 if j < 4 else 256) + (g + 1) * 128],
                                     rhs=val16[:, j * 129:(j + 1) * 129],
                                     start=(j == 0), stop=(j == NKT - 1))

        # ---------- normalize + output (f16; host upcasts) ----------
        rinv0 = stat.tile([128, 1], F32, tag="rinv0")
        nc.vector.reciprocal(rinv0[:], pav[0][:, 128:129])
        rinv1 = stat.tile([128, 1], F32, tag="rinv1")
        nc.vector.reciprocal(rinv1[:], pav[1][:, 128:129])
        nc.vector.tensor_scalar(osb[:, 0:128], pav[0][:, 0:128],
                                rinv0[:], None, op0=ALU.mult)
        nc.scalar.activation(osb[:, 128:256], pav[1][:, 0:128],
                             AF.Copy, scale=rinv1[:])
        # split output across the two HWDGE queues so the halves' DGE
        # stages overlap
        nc.sync.dma_start(out_d[:, 0:128], osb[:, 0:128])
        nc.scalar.dma_start(out_d[:, 128:256], osb[:, 128:256])


def _fix_writeback_sem(nc):
    """The kv_writeback(prepare_only) requires a user completion sem, which
    lands in on_update[0] -- the slot the SWDGE machinery treats as THE
    DMA-completion sem.  Tile assigned the prep a DMASW lane and made the
    final drain wait on that lane's semaphore, but never attached the
    update (the user sem occupies the slot).  Rewrite on_update[0] to the
    DMASW sem the drain expects (+16, the DMA-completion convention)."""
    has_kv = any(type(ins).__name__ == "InstKVWritebackAnt"
                 for f in nc.m.functions for blk in f.blocks
                 for ins in blk.instructions)
    if not has_kv:
        return
    target = None
    for f in nc.m.functions:
        for blk in f.blocks:
            for ins in blk.instructions:
                si = ins.sync_info
                if si is None:
                    continue
                for w in si.on_wait:
                    if w.ant_name and w.ant_name.startswith("DMASW"):
                        target = w
    assert target is not None, "no DMASW drain wait found"
    for f in nc.m.functions:
        for blk in f.blocks:
            for ins in blk.instructions:
                if type(ins).__name__ != "InstKVWritebackAnt":
                    continue
                si = ins.sync_info
                upd = mybir.SyncUpdate(sync_type="semaphore", id=target.id,
                                       ant_name=target.ant_name,
                                       update_mode="sem-add-imm",
                                       update_value=16)
                ins.sync_info = mybir.SyncInfo(
                    on_wait=list(si.on_wait),
                    on_update=[upd] + list(si.on_update)[1:])


def _drop_trailing_range_clear(nc):
    """This walrus rejects the raw EVENT_SEMAPHORE_RANGE_CLEAR InstISA
    ("ISA wrong length").  Tile emits exactly one, at the kernel tail, to
    recycle pool semaphores for later tiles — of which there are none, so
    dropping it is safe.  Verified: no later instruction waits on the range."""
    import re
    for f in nc.m.functions:
        for blk in f.blocks:
            insts = list(blk.instructions)
            keep, pending = [], []
            for ins in insts:
                if (type(ins).__name__ == "InstISA"
                        and "EVENT_SEMAPHORE_RANGE_CLEAR" in ins.concise()):
                    m = re.search(r"range_first=(\d+) range_last=(\d+)", ins.concise())
                    pending.append((ins, set(range(int(m.group(1)), int(m.group(2)) + 1))))
                    continue
                for _, rng in pending:
                    si = ins.sync_info
                    if si is not None:
                        used = {w.id for w in si.on_wait} | {u.id for u in si.on_update}
                        assert not (used & rng), (
                            f"range-clear removal unsafe: {ins.name} uses {used & rng}")
                keep.append(ins)
            blk.instructions = keep


def split_excess_waits(nc, max_waits=1):
    """This walrus rejects >1 sync-wait per instruction; move extras onto
    preceding no-ops on the same engine (engines issue in order, so a wait
    on an earlier instruction subsumes one on the original)."""
    _fix_writeback_sem(nc)
    _drop_trailing_range_clear(nc)
    n = 0
    for f in nc.m.functions:
        for blk in f.blocks:
            new_list = []
            for ins in blk.instructions:
                si = ins.sync_info
                if si is not None and len(si.on_wait) > max_waits:
                    waits = list(si.on_wait)
                    extra, keep = waits[:-max_waits], waits[-max_waits:]
                    for j in range(0, len(extra), max_waits):
                        nop = mybir.InstNoOp(
                            name=f"{ins.name}-ws{j}",
                            engine=ins.engine,
                            sync_info=mybir.SyncInfo(on_wait=extra[j:j + max_waits],
                                                     on_update=[]),
                            bass_nofuse=True,
                        )
                        new_list.append(nop)
                    ins.sync_info = mybir.SyncInfo(on_wait=keep,
                                                  on_update=list(si.on_update))
                    n += 1
                new_list.append(ins)
            blk.instructions = new_list
    return n


_CACHED_NC = None


def _get_nc():
    global _CACHED_NC
    if _CACHED_NC is None:
        nc = build_nc()
        split_excess_waits(nc)
        _CACHED_NC = nc
    return _CACHED_NC


def make_in_maps(query, key, value, vT, weight):
    query = np.asarray(query, np.float32)
    key = np.asarray(key, np.float32)
    value = np.asarray(value, np.float32)
    vT = np.asarray(vT, np.float32).reshape(E)
    weight = np.asarray(weight, np.float32)

    wqT = weight[:, :D].T                      # [D, E]
    wkT = weight[:, D:].T                      # [D, E]
    wkvt = C_LIN * (weight[:, D:].T @ vT)      # [D]
    coefs = np.stack([B1 * vT, 2 * B2 * vT, -4 * B2 * vT,
                      3 * B3 * vT, B3 * vT, -4 * B3 * vT, wkvt], axis=1)
    wqp = np.ascontiguousarray(
        np.concatenate([wqT, wkT, coefs], axis=1), np.float16)

    kT = [np.ascontiguousarray(key[b].T, np.float16) for b in range(B)]
    vpl = [np.ascontiguousarray(
        value[b].reshape(NKT, 128, VD).transpose(1, 0, 2).reshape(128, NKT * VD),
        np.float16) for b in range(B)]

    in_maps = []
    for c in range(N_CORES):
        b, qs = divmod(c, N_CORES // B)
        in_maps.append({
            "wqp": wqp,
            "qT": np.ascontiguousarray(query[b, qs * QSH:(qs + 1) * QSH].T,
                                       np.float16),
            "kT": kT[b],
            "v": vpl[b],
        })
    return in_maps


def kernel(query, key, value, vT, weight):
    nc = _get_nc()
    in_maps = make_in_maps(query, key, value, vT, weight)
    res = run_bass_kernel_spmd(nc, in_maps, core_ids=list(range(N_CORES)))
    out = np.empty((B, LQ, VD), np.float32)
    for c in range(N_CORES):
        b, qs = divmod(c, N_CORES // B)
        o = res.results[c]["out"].astype(np.float32)      # [128, 256]
        out[b, qs * QSH:(qs + 1) * QSH] = (
            o.reshape(128, 2, VD).swapaxes(0, 1).reshape(QSH, VD))
    return out
